# revision 17
# baseline (speedup 1.0000x reference)
"""Bilinear kernel for Trainium2 (8 NeuronCores, Bass/Tile).

out[i, j] = sum_{k,l} a[i,k] * w[j,k,l] * b[i,l] + bias[j]
with B=2048, K=L=512, H=512.

Strategy: shard H (the j dim) across 8 cores (64 j's each).
Per core, for each j:
  t_j[i, k] = sum_l b[i, l] * w[j, k, l]          (tensor engine, bf16,
       4 accumulating matmuls over l-chunks of 128; stationary = b^T tile,
       moving = w_j[l, k] tile, N=512 -> one PSUM bank)
  out[i, j] = bias[j] + sum_k a[i, k] * t_j[i, k]  (DVE tensor_mul into a
       PSUM prod tile, then ScalarE activation(Copy) with accum_out for the
       free-dim sum; bias added at the end with one small DVE add per i-tile;
       `a` stays fp32)

Weights/b are rounded to bf16 (matmul runs at 1 cycle/row vs fp32's 4);
all accumulation is fp32 (PSUM + DVE reduce).
"""

import numpy as np
import ml_dtypes

N_CORES = 8
B, K, L, H = 2048, 512, 512, 512
HJ = H // N_CORES      # j's per core
P = 128                # partitions
IT = B // P            # i-tiles
LC = L // P            # l-chunks

_BF16 = ml_dtypes.bfloat16

_prog_cache = {}


def build_nc(hj=HJ, it_count=IT, reps=1, t_bufs=3, prod_bufs=3, w_bufs=3,
             split_bt=False):
    """Build the per-core Bass/Tile program (SPMD: same program, per-core data).

    reps > 1 repeats the whole compute (same inputs/outputs) for HW-timing
    differencing; only the last rep's output is DMA'd out.
    """
    import concourse.bass as bass
    import concourse.tile as tile
    from concourse import bacc, mybir

    f32 = mybir.dt.float32
    bf16 = mybir.dt.bfloat16

    nc = bacc.Bacc(trn_type="TRN2")

    # Host-prearranged layouts (see kernel() below):
    #  wt[j, p, c, k]  = w[j_global, k, c*128 + p]   (bf16)
    #  bt[p, c, i]     = b[i, c*128 + p]             (bf16)
    #  ap[p, t, k]     = a[t*128 + p, k]             (fp32)
    #  biasr[p, j]     = bias[j_global]              (fp32, replicated over p)
    wt = nc.declare_dram_parameter("wt", [hj, P, LC, K], bf16, isOutput=False)
    bt = nc.declare_dram_parameter("bt", [P, LC, B], bf16, isOutput=False)
    ap = nc.declare_dram_parameter("ap", [P, it_count, K], f32, isOutput=False)
    biasr = nc.declare_dram_parameter("biasr", [P, hj], f32, isOutput=False)
    out = nc.declare_dram_parameter("out", [it_count, P, hj], f32, isOutput=True)

    with tile.TileContext(nc) as tc:
        with (
            tc.tile_pool(name="resident", bufs=1) as res_pool,
            tc.tile_pool(name="wpool", bufs=w_bufs) as wpool,
            tc.tile_pool(name="scratch", bufs=2) as scratch_pool,
            tc.tile_pool(name="psum", bufs=t_bufs, space=bass.MemorySpace.PSUM)
            as psum_pool,
            tc.tile_pool(name="psum_prod", bufs=prod_bufs,
                         space=bass.MemorySpace.PSUM) as prod_pool,
        ):
            # bt + the w stream go on the sync HWDGE ring; a/bias go on the
            # scalar ring so the 4MB a load doesn't delay the first matmuls.
            if split_bt:
                bt_chunks = []
                for c in range(LC):
                    t = res_pool.tile([P, B], bf16, tag=f"btc{c}", name=f"btc{c}")
                    nc.sync.dma_start(out=t[:], in_=bt[:, c, :])
                    bt_chunks.append(t)

                def bt_slice(c, it):
                    return bt_chunks[c][:, it * P:(it + 1) * P]
            else:
                bt_sb = res_pool.tile([P, LC, B], bf16, tag="bt")
                nc.sync.dma_start(out=bt_sb[:], in_=bt[:])

                def bt_slice(c, it):
                    return bt_sb[:, c, it * P:(it + 1) * P]
            a_sb = res_pool.tile([P, it_count, K], f32, tag="a")
            nc.scalar.dma_start(out=a_sb[:], in_=ap[:])
            bias_sb = res_pool.tile([P, hj], f32, tag="bias")
            nc.scalar.dma_start(out=bias_sb[:], in_=biasr[:])

            for rep in range(reps):
                out_sb = []
                for it in range(it_count):
                    out_sb.append(
                        res_pool.tile(
                            [P, hj], f32, tag=f"out{rep}_{it}", name=f"out{rep}_{it}"
                        )
                    )

                for j in range(hj):
                    w_sb = wpool.tile([P, LC, K], bf16, tag="w", name=f"w{rep}_{j}")
                    nc.sync.dma_start(out=w_sb[:], in_=wt[j])
                    for it in range(it_count):
                        t_ps = psum_pool.tile([P, K], f32, tag="t", name=f"t{rep}_{j}_{it}")
                        for c in range(LC):
                            nc.tensor.matmul(
                                t_ps[:],
                                bt_slice(c, it),
                                w_sb[:, c, :],
                                start=(c == 0),
                                stop=(c == LC - 1),
                            )
                        prod = prod_pool.tile([P, K], f32, tag="prod", name=f"p{rep}_{j}_{it}")
                        nc.vector.tensor_mul(prod[:], t_ps[:], a_sb[:, it, :])
                        scr = scratch_pool.tile([P, K], f32, tag="scr", name=f"s{rep}_{j}_{it}")
                        nc.scalar.activation(
                            out=scr[:],
                            in_=prod[:],
                            func=mybir.ActivationFunctionType.Copy,
                            accum_out=out_sb[it][:, j:j + 1],
                        )

                for it in range(it_count):
                    nc.vector.tensor_add(out_sb[it][:], out_sb[it][:], bias_sb[:])
                    if rep == reps - 1:
                        nc.sync.dma_start(out=out[it], in_=out_sb[it][:])

    nc.compile()
    return nc


def prep_inputs(a, b, weight, bias):
    """Host-side sharding + layout. Returns in_maps (one dict per core)."""
    a = np.asarray(a, dtype=np.float32)
    b = np.asarray(b, dtype=np.float32)
    weight = np.asarray(weight, dtype=np.float32)
    bias = np.asarray(bias, dtype=np.float32)

    # wt[j, p, c, k] = w[j, k, c*128+p]  (cast to bf16 first: halves copy volume)
    wt = weight.astype(_BF16).transpose(0, 2, 1)    # [H, L, K]
    wt = wt.reshape(H, LC, P, K)                    # [H, c, p, K]
    wt = np.ascontiguousarray(wt.transpose(0, 2, 1, 3))  # [H, p, c, K]

    # bt[p, c, i] = b[i, c*128+p]
    bt = b.T.reshape(LC, P, B).transpose(1, 0, 2)   # [p, c, i]
    bt = np.ascontiguousarray(bt).astype(_BF16)

    # ap[p, t, k] = a[t*128+p, k]
    apm = np.ascontiguousarray(a.reshape(IT, P, K).transpose(1, 0, 2))

    in_maps = []
    for c in range(N_CORES):
        jlo, jhi = c * HJ, (c + 1) * HJ
        in_maps.append({
            "wt": np.ascontiguousarray(wt[jlo:jhi]),
            "bt": bt,
            "ap": apm,
            "biasr": np.ascontiguousarray(
                np.broadcast_to(bias[jlo:jhi][None, :], (P, HJ))
            ),
        })
    return in_maps


def gather_output(results):
    """results: list (per core) of {"out": [IT, P, HJ] f32} -> [B, H] f32."""
    cols = []
    for c in range(N_CORES):
        o = np.asarray(results[c]["out"])         # [IT, P, HJ]
        cols.append(o.reshape(B, HJ))
    return np.concatenate(cols, axis=1)


def kernel(a, b, weight, bias):
    import time
    from concourse.bass_utils import run_bass_kernel_spmd

    if "nc" not in _prog_cache:
        _prog_cache["nc"] = build_nc()
    nc = _prog_cache["nc"]

    in_maps = prep_inputs(a, b, weight, bias)
    last_err = None
    for attempt in range(3):
        try:
            results = run_bass_kernel_spmd(
                nc, in_maps, core_ids=list(range(N_CORES))
            ).results
            return gather_output(results)
        except Exception as e:  # transient device/relay failures
            last_err = e
            time.sleep(10 * (attempt + 1))
    raise last_err
